# revision 14
# baseline (speedup 1.0000x reference)
"""BRITS GRU-cell recurrence on 8 Trainium2 NeuronCores.

Problem: B=8192 samples, T=256 timesteps, H=128 hidden. Data-parallel:
each core runs Bs=1024 samples through the full sequential recurrence.

Device layout per core: hidden state h lives in SBUF as [H=128 partitions,
Bs free]. All matmuls contract over H on the partition dim (fp32r = 1
cycle/row). Per-sample scalars (x_t, m_t, c_t, pred_t) are [1or2, Bs] rows.
I/O rows are staged in G-step blocks so DMAs are 32KB, not 4KB.

Per step:
  c      = tanh(clogit_prev + Wc_b)                  (ACT)
  omc    = (1-m)*c                                   (GPSIMD, into mxc row0)
  featpre= Wx (x) omc + Wx (x) mx                    (K=2 matmul; == Wx (x) x_imp)
  feat   = relu(featpre + Wx_b)                      (ACT)
  acc_rz = [Wih_r|Wih_z]@feat + [Whh_r|Whh_z]@h + [wm;b]@[m;1]  (PSUM [128,2048])
  rz     = sigmoid(acc_rz)                           (one ACT over 2048)
  i_n    = Wih_n@feat + [wm_n;b_ihn]@[m;1]           (PSUM)
  h_n    = Whh_n@h                                   (PSUM)
  rhn    = (h_n + b_hhn) * r                         (DVE scalar_tensor_tensor)
  n      = tanh(i_n + rhn)                           (DVE add + ACT)
  h      = n + z*(h-n)                               (GPSIMD sub/mul + DVE add)
  small  = [Wc_w|out_w].T @ h  -> [clogit; pred]     (M=2 matmul; DVE-copied to stage)
"""

import os
import sys
from contextlib import ExitStack

import numpy as np

for _p in ("/opt/trn_rl_repo", "/opt/pypackages"):
    if _p not in sys.path and os.path.isdir(_p):
        sys.path.append(_p)

import concourse.bass as bass
import concourse.bacc as bacc
import concourse.tile as tile
from concourse import mybir
from concourse.bass_utils import run_bass_kernel_spmd

B, T, H = 8192, 256, 128
NCORES = 8
BS = B // NCORES  # 1024 samples per core
NT = 2            # column tiles of 512
TN = BS // NT     # 512
G = 4             # steps per I/O block
F32 = mybir.dt.float32
F32R = mybir.dt.float32r


def r(ap):
    return ap.bitcast(F32R)


def build_program(t_steps=T, bs=BS):
    assert t_steps % G == 0
    nc = bacc.Bacc("TRN2", target_bir_lowering=False, debug=False)
    gbs = G * bs

    xm = nc.dram_tensor("xm", [t_steps, 2, bs], F32R, kind="ExternalInput").ap()
    wihT = nc.dram_tensor("wihT", [H, 3 * H], F32R, kind="ExternalInput").ap()
    whhT = nc.dram_tensor("whhT", [H, 3 * H], F32R, kind="ExternalInput").ap()
    wmb = nc.dram_tensor("wmb", [2, 3 * H], F32R, kind="ExternalInput").ap()
    wxw = nc.dram_tensor("wxw", [2, H], F32R, kind="ExternalInput").ap()
    wsmall = nc.dram_tensor("wsmall", [H, 2], F32R, kind="ExternalInput").ap()
    biases = nc.dram_tensor("biases", [H, 3], F32, kind="ExternalInput").ap()
    hz = nc.dram_tensor("hz", [H, bs], F32R, kind="ExternalInput").ap()
    mone = nc.dram_tensor("mone", [1, gbs], F32R, kind="ExternalInput").ap()
    opc = nc.dram_tensor("opc", [t_steps, 2, bs], F32, kind="ExternalOutput").ap()

    AF = mybir.ActivationFunctionType
    OP = mybir.AluOpType

    with tile.TileContext(nc) as tc, ExitStack() as ctx:
        const = ctx.enter_context(tc.tile_pool(name="const", bufs=1))
        work = ctx.enter_context(tc.tile_pool(name="work", bufs=2))
        ps_rz = ctx.enter_context(tc.tile_pool(name="ps_rz", bufs=1, space="PSUM"))
        ps_a = ctx.enter_context(tc.tile_pool(name="ps_a", bufs=1, space="PSUM"))
        ps_b = ctx.enter_context(tc.tile_pool(name="ps_b", bufs=1, space="PSUM"))

        # --- constants / persistent state ---
        w_ih = const.tile([H, 3 * H], F32R)
        nc.sync.dma_start(w_ih[:], wihT[:])
        w_hh = const.tile([H, 3 * H], F32R)
        nc.sync.dma_start(w_hh[:], whhT[:])
        w_mb = const.tile([2, 3 * H], F32R)
        nc.sync.dma_start(w_mb[:], wmb[:])
        w_xw = const.tile([2, H], F32R)
        nc.sync.dma_start(w_xw[:], wxw[:])
        w_sm = const.tile([H, 2], F32R)
        nc.sync.dma_start(w_sm[:], wsmall[:])
        bia = const.tile([H, 3], F32)
        nc.sync.dma_start(bia[:], biases[:])

        h = const.tile([H, bs], F32R)
        nc.sync.dma_start(h[:], hz[:])
        zrow = const.tile([1, bs], F32)
        nc.vector.memset(zrow[:], 0.0)

        # double-buffered per-block staging (persistent tiles)
        io = ctx.enter_context(tc.tile_pool(name="io", bufs=2))
        mrow_ab, mxc_ab, cp_ab = [], [], []
        for i_ in range(2):
            mt = const.tile([2, gbs], F32R, tag=f"mrow{i_}")
            nc.sync.dma_start(mt[1:2, :], mone[:])   # ones row for [m;1] rhs
            mrow_ab.append(mt)
            mxc_ab.append(const.tile([2, gbs], F32R, tag=f"mxc{i_}", name=f"mxc{i_}"))
            cp_ab.append(const.tile([2, gbs], F32, tag=f"cp{i_}", name=f"cp{i_}"))

        tc.strict_bb_all_engine_barrier()

        b_hhn = bia[:, 0:1]
        b_wx = bia[:, 1:2]
        b_wc = bia[0:1, 2:3]

        for t in range(t_steps):
            g = t % G
            blk = (t // G) % 2
            off = g * bs

            if g == 0:
                t0 = t
                nc.sync.dma_start(mrow_ab[blk][0:1, :], xm[t0:t0 + G, 0, :])
                nc.sync.dma_start(mxc_ab[blk][1:2, :], xm[t0:t0 + G, 1, :])
            mrow = mrow_ab[blk]
            mxc = mxc_ab[blk]
            so = slice(off, off + bs)

            # -- c = tanh(clogit + Wc_b), written into mxc row0 --
            if t == 0:
                cl_src = zrow[0:1, :]
            else:
                pblk = ((t - 1) // G) % 2
                po = ((t - 1) % G) * bs
                cl_src = cp_ab[pblk][0:1, po:po + bs]
            nc.scalar.activation(mxc[0:1, so], cl_src, AF.Tanh, bias=b_wc)

            # -- omc = c - m*c  (in place in mxc row0; f32r for matmul) --
            scr = io.tile([1, bs], F32, tag="scr")
            nc.gpsimd.tensor_mul(scr[:], mrow[0:1, so].bitcast(F32),
                                 mxc[0:1, so].bitcast(F32))
            nc.gpsimd.tensor_sub(mxc[0:1, so], mxc[0:1, so].bitcast(F32), scr[:])

            # -- featpre = Wx (x) omc + Wx (x) mx --
            featpre = ps_a.tile([H, bs], F32, tag="infp")
            for j in range(NT):
                s = slice(j * TN, (j + 1) * TN)
                nc.tensor.matmul(featpre[:, s], w_xw[:, :],
                                 mxc[0:2, off + j * TN:off + (j + 1) * TN],
                                 start=True, stop=True)
            feat = work.tile([H, bs], F32R, tag="feat")
            nc.scalar.activation(feat[:], featpre[:], AF.Relu, bias=b_wx)

            # -- gate matmuls --
            acc_rz = ps_rz.tile([H, 2 * bs], F32, tag="accrz")
            i_n = ps_a.tile([H, bs], F32, tag="infp")
            h_n = ps_b.tile([H, bs], F32, tag="hnsm")
            for j in range(NT):
                s = slice(j * TN, (j + 1) * TN)
                sz = slice(bs + j * TN, bs + (j + 1) * TN)
                sm = slice(off + j * TN, off + (j + 1) * TN)
                nc.tensor.matmul(acc_rz[:, s], w_ih[:, 0:H], feat[:, s],
                                 start=True, stop=False)
                nc.tensor.matmul(acc_rz[:, s], w_hh[:, 0:H], h[:, s],
                                 start=False, stop=False)
                nc.tensor.matmul(acc_rz[:, s], w_mb[:, 0:H], mrow[:, sm],
                                 start=False, stop=True)
                nc.tensor.matmul(acc_rz[:, sz], w_ih[:, H:2 * H], feat[:, s],
                                 start=True, stop=False)
                nc.tensor.matmul(acc_rz[:, sz], w_hh[:, H:2 * H], h[:, s],
                                 start=False, stop=False)
                nc.tensor.matmul(acc_rz[:, sz], w_mb[:, H:2 * H], mrow[:, sm],
                                 start=False, stop=True)
                nc.tensor.matmul(i_n[:, s], w_ih[:, 2 * H:], feat[:, s],
                                 start=True, stop=False)
                nc.tensor.matmul(i_n[:, s], w_mb[:, 2 * H:], mrow[:, sm],
                                 start=False, stop=True)
                nc.tensor.matmul(h_n[:, s], w_hh[:, 2 * H:], h[:, s],
                                 start=True, stop=True)

            # -- gates --
            rz = work.tile([H, 2 * bs], F32, tag="rz")
            nc.scalar.activation(rz[:], acc_rz[:], AF.Sigmoid)

            rhn = work.tile([H, bs], F32, tag="rhn")
            for j in range(NT):
                s = slice(j * TN, (j + 1) * TN)
                nc.vector.scalar_tensor_tensor(rhn[:, s], h_n[:, s], b_hhn,
                                               rz[:, s], OP.add, OP.mult)
            npre = work.tile([H, bs], F32, tag="npre")
            for j in range(NT):
                s = slice(j * TN, (j + 1) * TN)
                nc.vector.tensor_add(npre[:, s], i_n[:, s], rhn[:, s])
            n_sb = work.tile([H, bs], F32, tag="n_sb")
            nc.scalar.activation(n_sb[:], npre[:], AF.Tanh)

            # -- h = n + z*(h-n): sub/mul on gpsimd, final add on DVE --
            tmp = work.tile([H, bs], F32, tag="tmp")
            for j in range(NT):
                s = slice(j * TN, (j + 1) * TN)
                sz = slice(bs + j * TN, bs + (j + 1) * TN)
                nc.gpsimd.tensor_sub(tmp[:, s], h[:, s].bitcast(F32), n_sb[:, s])
                nc.gpsimd.tensor_mul(tmp[:, s], tmp[:, s], rz[:, sz])
            for j in range(NT):
                s = slice(j * TN, (j + 1) * TN)
                nc.vector.tensor_add(h[:, s], n_sb[:, s], tmp[:, s])

            # -- small matmul: [clogit; pred] --
            small = ps_b.tile([2, bs], F32, tag="hnsm")
            for j in range(NT):
                s = slice(j * TN, (j + 1) * TN)
                nc.tensor.matmul(small[:, s], w_sm[:, :], h[:, s],
                                 start=True, stop=True)
            for j in range(NT):
                s = slice(j * TN, (j + 1) * TN)
                nc.vector.tensor_copy(cp_ab[blk][0:2, off + j * TN:off + (j + 1) * TN],
                                      small[:, s])

            if g == G - 1:
                t0 = t - G + 1
                nc.sync.dma_start(opc[t0:t0 + G, 0, :], cp_ab[blk][0:1, :])
                nc.sync.dma_start(opc[t0:t0 + G, 1, :], cp_ab[blk][1:2, :])

    nc.compile()
    return nc


def make_in_maps(x_seq, m_seq, Wc_w, Wc_b, Wx_w, Wx_b, W_ih, W_hh, b_ih, b_hh,
                 out_w, out_b, t_steps=T, bs=BS, ncores=NCORES):
    f = np.float32
    wihT = np.ascontiguousarray(W_ih[:, :H].T, dtype=f)          # [128, 384]
    whhT = np.ascontiguousarray(W_hh.T, dtype=f)                 # [128, 384]
    wmb = np.empty((2, 3 * H), dtype=f)
    wmb[0] = W_ih[:, H]
    wmb[1, 0:H] = b_ih[0:H] + b_hh[0:H]
    wmb[1, H:2 * H] = b_ih[H:2 * H] + b_hh[H:2 * H]
    wmb[1, 2 * H:] = b_ih[2 * H:]
    wxw = np.ascontiguousarray(
        np.stack([Wx_w[:, 0], Wx_w[:, 0]], axis=0), dtype=f)     # [2, 128]
    wsmall = np.stack([Wc_w[0], out_w[0]], axis=1).astype(f)     # [128, 2]
    biases = np.zeros((H, 3), dtype=f)
    biases[:, 0] = b_hh[2 * H:]
    biases[:, 1] = Wx_b
    biases[0, 2] = Wc_b[0]

    xT = np.ascontiguousarray(x_seq.T, dtype=f)  # [T, B]
    mT = np.ascontiguousarray(m_seq.T, dtype=f)

    in_maps = []
    for i in range(ncores):
        sl = slice(i * bs, (i + 1) * bs)
        xmc = np.empty((t_steps, 2, bs), dtype=f)
        xmc[:, 0, :] = mT[:t_steps, sl]
        xmc[:, 1, :] = mT[:t_steps, sl] * xT[:t_steps, sl]
        in_maps.append({
            "xm": xmc, "wihT": wihT, "whhT": whhT, "wmb": wmb, "wxw": wxw,
            "wsmall": wsmall, "biases": biases,
            "hz": np.zeros((H, bs), dtype=f),
            "mone": np.ones((1, G * bs), dtype=f),
        })
    return in_maps


_CACHE = {}


def kernel(x_seq, m_seq, Wc_w, Wc_b, Wx_w, Wx_b, W_ih, W_hh, b_ih, b_hh,
           out_w, out_b):
    Wc_b = np.asarray(Wc_b)
    x_seq = np.asarray(x_seq, dtype=np.float32)
    m_seq = np.asarray(m_seq, dtype=np.float32)
    if "nc" not in _CACHE:
        _CACHE["nc"] = build_program()
    nc = _CACHE["nc"]
    in_maps = make_in_maps(x_seq, m_seq, np.asarray(Wc_w), np.asarray(Wc_b),
                           np.asarray(Wx_w), np.asarray(Wx_b), np.asarray(W_ih),
                           np.asarray(W_hh), np.asarray(b_ih), np.asarray(b_hh),
                           np.asarray(out_w), np.asarray(out_b))
    res = run_bass_kernel_spmd(nc, in_maps, list(range(NCORES)))
    preds = np.empty((B, T), dtype=np.float32)
    comps = np.empty((B, T), dtype=np.float32)
    for i in range(NCORES):
        o = res.results[i]["opc"]  # [T, 2, BS]
        sl = slice(i * BS, (i + 1) * BS)
        cl = o[:, 0, :].T  # clogit produced at step t = logit of c_{t+1}
        comps[sl, 1:] = np.tanh(cl[:, :-1] + np.float32(Wc_b[0]))
        comps[sl, 0] = np.tanh(np.float32(Wc_b[0]))
        preds[sl, :] = o[:, 1, :].T + np.float32(out_b[0])
    return preds, comps
